# revision 1
# baseline (speedup 1.0000x reference)
"""Trainium2 Bass kernel for nn_AttentionHead (B=8, N=2048, D=512, d=64).

Reference semantics (faithful to the torch original):
    K = key_input   @ W_key        # note: W_key used for Q, K AND V
    Q = query_input @ W_key
    V = value_input @ W_key
    S = Q @ K^T / sqrt(512)        # scaled by INPUT dim, not head dim
    S = mask(padding), causal-mask if masked_attention
    out = softmax(S) @ V

Sharding: pure data parallelism over batch — core b computes batch element b.
No collectives. Host-side prep is layout only (transpose + bf16 cast); every
FLOP of the math runs on-device.

Device algorithm (per core):
  - inputs stream in n-slices of 512 so attention on early q-blocks starts
    after ~1/4 of the DMA
  - QT/KT [64->dup 128, 2048] projections on TensorE (W chunks stationary);
    QT/KT are duplicated onto partitions 64-127 so pairs of S matmuls
    (contraction only 64-deep) run concurrently in disjoint PE row groups
  - V transposed back to natural [128, 65] tiles via PE-transpose with a ones
    column appended (row-sums of P come free as row 64 of the PV matmul)
  - per k-chunk j: S.T tile [k=128, q<=512] = KT_j.T @ QT_qb (exact-causal
    widths); exp via ACT straight PSUM->SBUF bf16 with 1/sqrt(512) folded in;
    diagonal 128x128 blocks masked by affine_select on GpSimd
  - O.T [65, q] += V'_j.T @ P.T accumulated in PSUM over k-chunks
  - epilogue per q-block: PE-transpose O.T, divide rows by the sums column,
    DMA out f32
"""

import math

import numpy as np
import ml_dtypes

import concourse.bass as bass
import concourse.tile as tile
from concourse import bacc, mybir
from concourse import masks
from concourse.bass_utils import run_bass_kernel_spmd

P = 128            # partitions / k-chunk size
N = 2048           # sequence length
D = 512            # embedding dim
DH = 64            # head dim
EC = D // P        # 4 e-chunks for the projection contraction
KC = N // P        # 16 k-chunks
QW = 512           # q block width
NQB = N // QW      # 4 q blocks / n slices
SCALE = 1.0 / math.sqrt(float(D))

BF16 = mybir.dt.bfloat16
F32 = mybir.dt.float32

_BUILD_CACHE = {}

# structural knobs (tuned via TimelineSim sweeps; defaults = best known)
OPTS = {
    "sbufs": 2,            # s psum pool buffers
    "jbufs": 2,            # proj/transpose psum pool buffers
    "dma_mode": "fine_split",  # input DMA granularity/queue split
    "mask_dve": True,      # diag causal mask: DVE tri-multiply vs gpsimd affine
    "out_batch": True,     # batch output DMA per q-block
    "order": "phases",     # attention emission order: phases | chained | trail
    "pe_warm": 20,         # dummy matmuls at t=0 to lift the HAM clock gate
}


def _ensure_ntff_hook():
    """Install the antenv.axon_hooks shim so trace=True works under axon."""
    try:
        import antenv.axon_hooks  # noqa: F401
        return
    except ImportError:
        pass
    import sys
    import types

    try:
        from trn_agent_boot.trn_boot import _ntff_profile_via_ctypes
        hook = _ntff_profile_via_ctypes("/opt/axon/libaxon_pjrt.so")
    except Exception:
        hook = None
    mod = types.ModuleType("antenv.axon_hooks")
    state = {"hook": hook}
    mod.get_axon_ntff_profile_hook = lambda: state["hook"]
    mod.set_axon_ntff_profile_hook = lambda h: state.update(hook=h)
    sys.modules["antenv.axon_hooks"] = mod
    import antenv

    antenv.axon_hooks = mod


def _build(causal: bool, has_padding: bool):
    nc = bacc.Bacc("TRN2", target_bir_lowering=False, debug=False, num_devices=8)

    xq_d = nc.dram_tensor("xq_t", [D, N], BF16, kind="ExternalInput")
    xk_d = nc.dram_tensor("xk_t", [D, N], BF16, kind="ExternalInput")
    xv_d = nc.dram_tensor("xv_t", [D, N], BF16, kind="ExternalInput")
    # w is host-duplicated [D, 2*DH] = [W | W] so the Q/K projections emit
    # [128, q] tiles whose partition halves are copies — S matmul pairs can
    # then row-pack into disjoint PE row groups with no cross-partition copy.
    w_d = nc.dram_tensor("w", [D, 2 * DH], BF16, kind="ExternalInput")
    if has_padding:
        km_d = nc.dram_tensor("kmask", [KC, P], F32, kind="ExternalInput")
    out_d = nc.dram_tensor("out", [N, DH], F32, kind="ExternalOutput")

    with tile.TileContext(nc) as tc:
        with (
            tc.tile_pool(name="const", bufs=1) as cpool,
            tc.tile_pool(name="x", bufs=12) as xpool,
            tc.tile_pool(name="big", bufs=1) as bigpool,
            tc.tile_pool(name="p", bufs=8) as ppool,
            tc.tile_pool(name="epi", bufs=2) as epipool,
            tc.tile_pool(name="o", bufs=4, space="PSUM") as opool,
            tc.tile_pool(name="s", bufs=OPTS["sbufs"], space="PSUM") as spool,
            tc.tile_pool(name="j", bufs=OPTS["jbufs"], space="PSUM") as jpool,
        ):
            # --- ACT warmup (hide exp table load behind the DMA window) ---
            warm = cpool.tile([P, 1], F32)
            nc.vector.memset(warm[:], 0.0)
            nc.scalar.activation(warm[:], warm[:], mybir.ActivationFunctionType.Exp)

            # --- PE warmup: HAM clock-gates the PE array to 1.2 GHz until it
            # sees ~3.4us of sustained matmul activity; spin dummy matmuls
            # during the DMA window so real work runs at 2.4 GHz ---
            if OPTS["pe_warm"]:
                wjunk = cpool.tile([P, P], BF16)
                nc.vector.memset(wjunk[:], 0.25)
                wpsum = opool.tile([DH + 1, QW], F32, tag="o", name="warmps")
                for _ in range(OPTS["pe_warm"]):
                    nc.tensor.matmul(
                        wpsum[:, :P], wjunk[:, :DH + 1], wjunk[:, :P],
                        start=True, stop=True, skip_group_check=True,
                    )

            ident = cpool.tile([P, P], F32)
            masks.make_identity(nc, ident[:])
            # upper-triangular (incl diag) 0/1 mask in [k, q] coords for the
            # causal diagonal blocks; multiply on DVE (gpsimd's slow semaphore
            # handling would sit in the exp->PV chain otherwise)
            tri = cpool.tile([P, P], BF16)
            masks.make_upper_triangular(nc, tri[:], val=1.0, diag=True)

            w_sb = cpool.tile([P, EC, 2 * DH], BF16)
            nc.sync.dma_start(w_sb[:], w_d.ap().rearrange("(c p) d -> p c d", p=P))
            if has_padding:
                km_sb = cpool.tile([P, KC], F32)
                nc.sync.dma_start(km_sb[:], km_d.ap().transpose([1, 0]))

            # --- input DMAs: issue spread over queues
            # (DMA issue is ~800ns serial per op on the issuing engine) ---
            x_sb = {}
            mode = OPTS["dma_mode"]
            if mode == "half":
                nch, chw = 2, 2 * QW
                engs = {"q": nc.sync, "k": nc.sync, "v": nc.gpsimd}
            elif mode == "fine_split":
                nch, chw = NQB, QW
                engs = {"q": nc.sync, "k": nc.sync, "v": nc.gpsimd}
            else:  # "fine"
                nch, chw = NQB, QW
                engs = {"q": nc.sync, "k": nc.sync, "v": nc.sync}
            for nh in range(nch):
                for tname, xd in (("q", xq_d), ("k", xk_d), ("v", xv_d)):
                    t = xpool.tile([P, EC, chw], BF16, tag="x")
                    engs[tname].dma_start(
                        t[:],
                        xd.ap()[:, nh * chw:(nh + 1) * chw].rearrange(
                            "(c p) q -> p c q", p=P
                        ),
                    )
                    x_sb[(tname, nh)] = t

            qt = bigpool.tile([P, N], BF16, tag="qt")   # rows 0-63 QT, 64-127 dup
            kt = bigpool.tile([P, N], BF16, tag="kt")
            vt = bigpool.tile([DH, N], F32, tag="vt")
            v_sb = bigpool.tile([P, KC, DH + 1], BF16, tag="vn")

            # --- projections + V-natural, per n-slice ---
            for ns in range(NQB):
                sl = slice(ns * QW, (ns + 1) * QW)
                if OPTS["dma_mode"] == "half":
                    nh, qo = ns // 2, (ns % 2) * QW
                else:
                    nh, qo = ns, 0
                for tname in ("q", "k", "v"):
                    wide = tname != "v"   # q/k project through [W|W] -> M=128
                    m = P if wide else DH
                    ps = jpool.tile([P, QW], F32, tag="j")
                    for c in range(EC):
                        nc.tensor.matmul(
                            ps[:m, :],
                            w_sb[:, c, :m],
                            x_sb[(tname, nh)][:, c, qo:qo + QW],
                            start=(c == 0),
                            stop=(c == EC - 1),
                        )
                    if tname == "q":
                        nc.vector.tensor_copy(qt[:, sl], ps[:])
                    elif tname == "k":
                        nc.vector.tensor_copy(kt[:, sl], ps[:])
                    else:
                        nc.vector.tensor_copy(vt[:, sl], ps[:DH, :])
                # V natural tiles for this n-slice: PE transpose + ones column
                vtp = jpool.tile([P, NQB, DH + 1], F32, tag="j")
                for i in range(NQB):
                    j = ns * NQB + i
                    nc.tensor.transpose(
                        vtp[:, i, :DH], vt[:, j * P:(j + 1) * P], ident[:DH, :DH]
                    )
                nc.vector.memset(vtp[:, :, DH], 1.0)
                nc.vector.tensor_copy(v_sb[:, ns * NQB:(ns + 1) * NQB, :], vtp[:])

            # --- attention: k-chunk pairs (row-packed S), q-blocks inner ---
            o_tiles = [
                opool.tile([DH + 1, QW], F32, tag="o", name=f"o{qb}")
                for qb in range(NQB)
            ]
            def emit_s(j, qb, idx, p_tiles):
                base = DH * idx
                q_off = max(0, j * P - qb * QW) if causal else 0
                width = QW - q_off
                s_ps = spool.tile([P, QW], F32, tag="s", name=f"s{j}_{qb}")
                nc.tensor.matmul(
                    s_ps[:, :width],
                    kt[base:base + DH, j * P:(j + 1) * P],
                    qt[base:base + DH, qb * QW + q_off:(qb + 1) * QW],
                    start=True,
                    stop=True,
                )
                p_sb = ppool.tile([P, QW], BF16, tag="p", name=f"p{j}_{qb}")
                nc.scalar.activation(
                    p_sb[:, :width],
                    s_ps[:, :width],
                    mybir.ActivationFunctionType.Exp,
                    scale=SCALE,
                )
                if causal and qb == j // NQB:
                    # diagonal block at cols [0,128): keep q_loc >= k_loc
                    if OPTS["mask_dve"]:
                        nc.vector.tensor_mul(p_sb[:, :P], p_sb[:, :P], tri[:])
                    else:
                        nc.gpsimd.affine_select(
                            out=p_sb[:, :P],
                            in_=p_sb[:, :P],
                            compare_op=mybir.AluOpType.is_ge,
                            fill=0.0,
                            base=0,
                            pattern=[[1, P]],
                            channel_multiplier=-1,
                        )
                if has_padding:
                    nc.vector.tensor_scalar_mul(
                        p_sb[:, :width], p_sb[:, :width], km_sb[:, j:j + 1]
                    )
                p_tiles[(j, qb)] = (p_sb, q_off, width)

            def emit_pv(j, qb, p_tiles):
                p_sb, q_off, width = p_tiles.pop((j, qb))
                j_last = ((QW // P) * (qb + 1) - 1) if causal else (KC - 1)
                nc.tensor.matmul(
                    o_tiles[qb][:, q_off:QW],
                    v_sb[:, j, :],
                    p_sb[:, :width],
                    start=(j == 0),
                    stop=(j == j_last),
                )

            for tp in range(KC // 2):
                js = (2 * tp, 2 * tp + 1)
                qb_lo = (js[0] // NQB) if causal else 0
                p_tiles = {}
                order = OPTS["order"]
                if order == "phases":
                    for qb in range(qb_lo, NQB):
                        for idx, j in enumerate(js):
                            emit_s(j, qb, idx, p_tiles)
                    for j in js:
                        for qb in range(qb_lo, NQB):
                            emit_pv(j, qb, p_tiles)
                elif order == "chained":
                    for qb in range(qb_lo, NQB):
                        for idx, j in enumerate(js):
                            emit_s(j, qb, idx, p_tiles)
                        for j in js:
                            emit_pv(j, qb, p_tiles)
                else:  # trail: PV lags S by one q-block
                    for qb in range(qb_lo, NQB):
                        for idx, j in enumerate(js):
                            emit_s(j, qb, idx, p_tiles)
                        if qb > qb_lo:
                            for j in js:
                                emit_pv(j, qb - 1, p_tiles)
                    for j in js:
                        emit_pv(j, NQB - 1, p_tiles)

                # epilogue for q-blocks completed by this pair
                done_qb = []
                if causal:
                    if js[1] % (QW // P) == QW // P - 1:
                        done_qb = [js[1] // (QW // P)]
                elif tp == KC // 2 - 1:
                    done_qb = list(range(NQB))
                for qb in done_qb:
                    oT = epipool.tile([DH + 1, QW], F32, tag="ot")
                    nc.vector.tensor_copy(oT[:], o_tiles[qb][:])
                    etp = jpool.tile([P, NQB, DH + 1], F32, tag="j")
                    for i in range(NQB):
                        nc.tensor.transpose(
                            etp[:, i, :], oT[:, i * P:(i + 1) * P],
                            ident[:DH + 1, :DH + 1],
                        )
                    recip = epipool.tile([P, NQB], F32, tag="recip")
                    nc.vector.reciprocal(recip[:], etp[:, :, DH])
                    if OPTS["out_batch"]:
                        o_sb = epipool.tile([P, NQB, DH], F32, tag="osb")
                        for i in range(NQB):
                            nc.vector.tensor_scalar_mul(
                                o_sb[:, i, :], etp[:, i, :DH], recip[:, i:i + 1]
                            )
                        nc.sync.dma_start(
                            out_d.ap()[qb * QW:(qb + 1) * QW, :].rearrange(
                                "(i p) d -> p i d", p=P
                            ),
                            o_sb[:],
                        )
                    else:
                        for i in range(NQB):
                            o_sb = epipool.tile([P, DH], F32, tag="osb")
                            nc.vector.tensor_scalar_mul(
                                o_sb[:], etp[:, i, :DH], recip[:, i:i + 1]
                            )
                            row = (qb * NQB + i) * P
                            nc.sync.dma_start(
                                out_d.ap()[row:row + P, :], o_sb[:]
                            )

    nc.compile()
    return nc


def _get(causal: bool, has_padding: bool):
    key = (causal, has_padding)
    if key not in _BUILD_CACHE:
        _BUILD_CACHE[key] = _build(causal, has_padding)
    return _BUILD_CACHE[key]


def run(key_input, query_input, value_input, padding_mask, masked_attention,
        W_key, W_query=None, W_value=None, trace=False, **_ignored):
    key_input = np.asarray(key_input, dtype=np.float32)
    query_input = np.asarray(query_input, dtype=np.float32)
    value_input = np.asarray(value_input, dtype=np.float32)
    padding_mask = np.asarray(padding_mask)
    W_key = np.asarray(W_key, dtype=np.float32)

    B = key_input.shape[0]
    causal = bool(int(np.asarray(masked_attention)))
    has_padding = bool(padding_mask.any())
    nc = _get(causal, has_padding)

    bf = ml_dtypes.bfloat16
    w_b = np.ascontiguousarray(
        np.concatenate([W_key, W_key], axis=1).astype(bf)
    )
    in_maps = []
    for b in range(B):
        m = {
            "xq_t": np.ascontiguousarray(query_input[b].T.astype(bf)),
            "xk_t": np.ascontiguousarray(key_input[b].T.astype(bf)),
            "xv_t": np.ascontiguousarray(value_input[b].T.astype(bf)),
            "w": w_b,
        }
        if has_padding:
            # multiplicative key mask in [KC, P] layout: 0 where padded
            km = (~padding_mask[b].reshape(N)).astype(np.float32)
            m["kmask"] = np.ascontiguousarray(km.reshape(KC, P))
        in_maps.append(m)

    if trace:
        _ensure_ntff_hook()
    res = run_bass_kernel_spmd(nc, in_maps, core_ids=list(range(B)), trace=trace)
    out = np.stack([np.asarray(res.results[b]["out"]) for b in range(B)], axis=0)
    return out.astype(np.float32), res


def kernel(**inputs) -> np.ndarray:
    out, _ = run(**inputs)
    return out



# revision 6
# speedup vs baseline: 1.0799x; 1.0799x over previous
"""Trainium2 Bass kernel for nn_AttentionHead (B=8, N=2048, D=512, d=64).

Reference semantics (faithful to the torch original):
    K = key_input   @ W_key        # note: W_key used for Q, K AND V
    Q = query_input @ W_key
    V = value_input @ W_key
    S = Q @ K^T / sqrt(512)        # scaled by INPUT dim, not head dim
    S = mask(padding), causal-mask if masked_attention
    out = softmax(S) @ V

Sharding: pure data parallelism over batch — core b computes batch element b.
No collectives. Host-side prep is layout only (transpose + dtype cast); every
FLOP of the math runs on-device.

Device algorithm (per core), v2:
  - xq/xk stream in fp8e4 (W_qk prescaled x16 host-side, descale folded into
    the exp scale); xv stays bf16 to protect output precision
  - host packs inputs in the exact SBUF tile layout [slice*p, chunk, q] so
    every DMA line is contiguous; DMA issue is spread over 4 engine queues
    with slice-0/1 q/k prioritized
  - q-block-outer attention: per 512-wide q-block, k-chunks processed in
    row-packed pairs (two 64-deep S matmuls concurrently in disjoint PE row
    groups) writing one [128, 1024] PSUM tile; ONE wide exp per pair on ACT
    (amortizes the ~293ns ACTIVATE overhead)
  - diagonal k-chunks compute S full-width (free under row-pairing) so the
    wide exp read is contiguous; only 128-wide diagonal blocks get the
    upper-triangular mask multiply on DVE; PV uses exact causal widths
  - O.T [65, q] accumulated in PSUM over k-chunks (ones column appended to
    V-natural gives softmax denominators as row 64); per-q-block epilogue:
    PE-transpose, reciprocal, scale, batched f32 DMA out
  - PSUM: 2x [128,1024] S (4 banks) + 2x [65,512] O (2) + 2x [128,512]
    proj/transpose (2) = 8 banks
"""

import math

import numpy as np
import ml_dtypes

import concourse.bass as bass
import concourse.tile as tile
from concourse import bacc, mybir
from concourse import masks
from concourse.bass_utils import run_bass_kernel_spmd

P = 128            # partitions / k-chunk size
N = 2048           # sequence length
D = 512            # embedding dim
DH = 64            # head dim
EC = D // P        # 4 e-chunks for the projection contraction
KC = N // P        # 16 k-chunks
QW = 512           # q block width
NQB = N // QW      # 4 q blocks / n slices
WS = 16.0          # host-side W_qk prescale (fp8 range use)
SCALE = 1.0 / math.sqrt(float(D))
EXP_SCALE = SCALE / (WS * WS)

BF16 = mybir.dt.bfloat16
FP8 = mybir.dt.float8e4
F32 = mybir.dt.float32

_BUILD_CACHE = {}

OPTS = {
    "pe_warm": 18,     # dummy matmuls at t=0 to lift the HAM clock gate
    "ppool": 4,        # p_sb wide-tile buffers (ACT run-ahead depth)
}


def _ensure_ntff_hook():
    """Install the antenv.axon_hooks shim so trace=True works under axon."""
    try:
        import antenv.axon_hooks  # noqa: F401
        return
    except ImportError:
        pass
    import sys
    import types

    try:
        from trn_agent_boot.trn_boot import _ntff_profile_via_ctypes
        hook = _ntff_profile_via_ctypes("/opt/axon/libaxon_pjrt.so")
    except Exception:
        hook = None
    mod = types.ModuleType("antenv.axon_hooks")
    state = {"hook": hook}
    mod.get_axon_ntff_profile_hook = lambda: state["hook"]
    mod.set_axon_ntff_profile_hook = lambda h: state.update(hook=h)
    sys.modules["antenv.axon_hooks"] = mod
    import antenv

    antenv.axon_hooks = mod


def _build(causal: bool, has_padding: bool):
    nc = bacc.Bacc("TRN2", target_bir_lowering=False, debug=False, num_devices=8)

    # inputs prepacked host-side in SBUF tile layout [(slice p), chunk, qw]
    xq_d = nc.dram_tensor("xq", [NQB * P, EC, QW], FP8, kind="ExternalInput")
    xk_d = nc.dram_tensor("xk", [NQB * P, EC, QW], FP8, kind="ExternalInput")
    xv_d = nc.dram_tensor("xv", [NQB * P, EC, QW], BF16, kind="ExternalInput")
    # wqk is host-duplicated [W|W]*WS so Q/K projections emit [128, q] tiles
    # whose partition halves are copies — S matmul pairs then row-pack into
    # disjoint PE row groups with no cross-partition copy.
    wqk_d = nc.dram_tensor("wqk", [P, EC, 2 * DH], FP8, kind="ExternalInput")
    wv_d = nc.dram_tensor("wv", [P, EC, DH], BF16, kind="ExternalInput")
    if has_padding:
        km_d = nc.dram_tensor("kmask", [KC, P], F32, kind="ExternalInput")
    out_d = nc.dram_tensor("out", [N, DH], F32, kind="ExternalOutput")

    with tile.TileContext(nc) as tc:
        with (
            tc.tile_pool(name="const", bufs=1) as cpool,
            tc.tile_pool(name="x", bufs=16) as xpool,
            tc.tile_pool(name="big", bufs=1) as bigpool,
            tc.tile_pool(name="p", bufs=OPTS["ppool"]) as ppool,
            tc.tile_pool(name="epi", bufs=2) as epipool,
            tc.tile_pool(name="o", bufs=2, space="PSUM") as opool,
            tc.tile_pool(name="s", bufs=2, space="PSUM") as spool,
            tc.tile_pool(name="j", bufs=2, space="PSUM") as jpool,
        ):
            # --- ACT warmup: load the exp table during the DMA window ---
            warm = cpool.tile([P, 1], F32)
            nc.vector.memset(warm[:], 0.0)
            nc.scalar.activation(warm[:], warm[:], mybir.ActivationFunctionType.Exp)

            # consts emitted before any DMA issue so their engines (vector
            # memset, gpsimd affine_select) aren't stuck behind 0.7-1us
            # dma_start issue slots
            wjunk = cpool.tile([P, P], BF16)
            nc.vector.memset(wjunk[:], 0.25)
            ident = cpool.tile([P, P], F32)
            masks.make_identity(nc, ident[:])
            # upper-triangular (incl diag) 0/1 mask in [k, q] coords for the
            # causal diagonal blocks
            tri = cpool.tile([P, P], BF16)
            masks.make_upper_triangular(nc, tri[:], val=1.0, diag=True)

            # --- weights + input DMAs, spread across engine queues with
            # slice-0/1 q/k first (each dma_start costs ~0.7-1us of issue
            # time on its engine) ---
            wqk_sb = cpool.tile([P, EC, 2 * DH], FP8)
            wv_sb = cpool.tile([P, EC, DH], BF16)
            nc.gpsimd.dma_start(wqk_sb[:], wqk_d.ap())
            nc.gpsimd.dma_start(wv_sb[:], wv_d.ap())
            if has_padding:
                km_sb = cpool.tile([P, KC], F32)
                nc.sync.dma_start(km_sb[:], km_d.ap().transpose([1, 0]))

            xq_sb, xk_sb, xv_sb = {}, {}, {}
            for s in range(NQB):
                xq_sb[s] = xpool.tile([P, EC, QW], FP8, tag="x", name=f"xq{s}")
                xk_sb[s] = xpool.tile([P, EC, QW], FP8, tag="x", name=f"xk{s}")
                # v split in two half-tiles so two queues pull in parallel
                xv_sb[(s, 0)] = xpool.tile([P, EC // 2, QW], BF16, tag="x",
                                           name=f"xv{s}a")
                xv_sb[(s, 1)] = xpool.tile([P, EC // 2, QW], BF16, tag="x",
                                           name=f"xv{s}b")

            def dma_x(eng, t, dram, s, half=None):
                rows = dram.ap()[s * P:(s + 1) * P]
                if half is None:
                    eng.dma_start(t[:], rows)
                else:
                    eng.dma_start(
                        t[:], rows[:, half * (EC // 2):(half + 1) * (EC // 2), :]
                    )

            # priority: q0 k0 q1 k1 | v0 | v1 | slice2 | slice3
            # (only SP/Activation/gpsimd can issue DMAs; scalar stops early
            # so the exp stream isn't delayed)
            dma_x(nc.scalar, xq_sb[0], xq_d, 0)
            dma_x(nc.sync, xk_sb[0], xk_d, 0)
            dma_x(nc.gpsimd, xk_sb[1], xk_d, 1)
            dma_x(nc.sync, xq_sb[1], xq_d, 1)
            dma_x(nc.scalar, xv_sb[(0, 0)], xv_d, 0, half=0)
            dma_x(nc.sync, xv_sb[(0, 1)], xv_d, 0, half=1)
            dma_x(nc.scalar, xv_sb[(1, 0)], xv_d, 1, half=0)
            dma_x(nc.gpsimd, xv_sb[(1, 1)], xv_d, 1, half=1)
            dma_x(nc.sync, xq_sb[2], xq_d, 2)
            dma_x(nc.gpsimd, xk_sb[2], xk_d, 2)
            dma_x(nc.sync, xv_sb[(2, 0)], xv_d, 2, half=0)
            dma_x(nc.gpsimd, xv_sb[(2, 1)], xv_d, 2, half=1)
            dma_x(nc.sync, xq_sb[3], xq_d, 3)
            dma_x(nc.gpsimd, xk_sb[3], xk_d, 3)
            dma_x(nc.sync, xv_sb[(3, 0)], xv_d, 3, half=0)
            dma_x(nc.gpsimd, xv_sb[(3, 1)], xv_d, 3, half=1)

            # --- PE warmup: HAM clock-gates the PE array to 1.2 GHz until
            # ~3.4us of sustained matmul activity; spin dummy matmuls during
            # the DMA window so real work runs at 2.4 GHz ---
            if OPTS["pe_warm"]:
                wps = jpool.tile([P, QW], F32, tag="j", name="warmps")
                for _ in range(OPTS["pe_warm"]):
                    nc.tensor.matmul(
                        wps[:, :P], wjunk[:], wjunk[:],
                        start=True, stop=True, skip_group_check=True,
                    )

            qt = bigpool.tile([P, N], BF16, tag="qt")   # rows 0-63 QT, 64-127 dup
            kt = bigpool.tile([P, N], BF16, tag="kt")
            vt = bigpool.tile([DH, N], F32, tag="vt")
            v_sb = bigpool.tile([P, KC, DH + 1], BF16, tag="vn")

            def proj_qk(s):
                sl = slice(s * QW, (s + 1) * QW)
                for tname, x_t, big in (("q", xq_sb[s], qt), ("k", xk_sb[s], kt)):
                    ps = jpool.tile([P, QW], F32, tag="j", name=f"{tname}p{s}")
                    for c in range(EC):
                        nc.tensor.matmul(
                            ps[:],
                            wqk_sb[:, c, :],
                            x_t[:, c, :],
                            start=(c == 0),
                            stop=(c == EC - 1),
                        )
                    nc.vector.tensor_copy(big[:, sl], ps[:])

            def proj_v(s):
                sl = slice(s * QW, (s + 1) * QW)
                ps = jpool.tile([P, QW], F32, tag="j", name=f"vp{s}")
                for c in range(EC):
                    nc.tensor.matmul(
                        ps[:DH, :],
                        wv_sb[:, c, :],
                        xv_sb[(s, c // 2)][:, c % 2, :],
                        start=(c == 0),
                        stop=(c == EC - 1),
                    )
                nc.vector.tensor_copy(vt[:, sl], ps[:DH, :])
                # V natural tiles: PE transpose + ones column (row-sums of P
                # come free as row 64 of the PV matmul)
                vtp = jpool.tile([P, NQB, DH + 1], F32, tag="j", name=f"vt{s}")
                for i in range(NQB):
                    j = s * NQB + i
                    nc.tensor.transpose(
                        vtp[:, i, :DH], vt[:, j * P:(j + 1) * P], ident[:DH, :DH]
                    )
                nc.vector.memset(vtp[:, :, DH], 1.0)
                nc.vector.tensor_copy(v_sb[:, s * NQB:(s + 1) * NQB, :], vtp[:])

            # --- attention, q-block outer; k-chunk pairs row-packed ---
            def emit_s_pair(qb, t, p_tiles):
                """S for k-chunks (2t, 2t+1) over q-block qb: two concurrent
                row-group matmuls into one wide PSUM tile, one wide exp."""
                j0, j1 = 2 * t, 2 * t + 1
                s_ps = spool.tile([P, 2 * QW], F32, tag="s", name=f"s{qb}_{t}")
                # exp reads contiguously from q_off0; j1 computes full width
                # so no unwritten PSUM is read
                q_off0 = max(0, j0 * P - qb * QW) if causal else 0
                nc.tensor.matmul(
                    s_ps[:, q_off0:QW],
                    kt[0:DH, j0 * P:(j0 + 1) * P],
                    qt[0:DH, qb * QW + q_off0:(qb + 1) * QW],
                    start=True, stop=True,
                )
                nc.tensor.matmul(
                    s_ps[:, QW:],
                    kt[DH:P, j1 * P:(j1 + 1) * P],
                    qt[DH:P, qb * QW:(qb + 1) * QW],
                    start=True, stop=True,
                )
                p_sb = ppool.tile([P, 2 * QW], BF16, tag="p", name=f"p{qb}_{t}")
                nc.scalar.activation(
                    p_sb[:, q_off0:],
                    s_ps[:, q_off0:],
                    mybir.ActivationFunctionType.Exp,
                    scale=EXP_SCALE,
                )
                if causal:
                    for idx, j in enumerate((j0, j1)):
                        if j // NQB == qb:
                            # diagonal 128x128 block: keep q_loc >= k_loc
                            lo = idx * QW + (j % NQB) * P
                            nc.vector.tensor_mul(
                                p_sb[:, lo:lo + P], p_sb[:, lo:lo + P], tri[:]
                            )
                if has_padding:
                    for idx, j in enumerate((j0, j1)):
                        off = max(0, j * P - qb * QW) if causal else 0
                        nc.vector.tensor_scalar_mul(
                            p_sb[:, idx * QW + off:(idx + 1) * QW],
                            p_sb[:, idx * QW + off:(idx + 1) * QW],
                            km_sb[:, j:j + 1],
                        )
                p_tiles[t] = p_sb

            def emit_pv(qb, t, o_ps, p_tiles, j_last):
                p_sb = p_tiles.pop(t)
                for idx, j in enumerate((2 * t, 2 * t + 1)):
                    q_off = max(0, j * P - qb * QW) if causal else 0
                    nc.tensor.matmul(
                        o_ps[:, q_off:QW],
                        v_sb[:, j, :],
                        p_sb[:, idx * QW + q_off:(idx + 1) * QW],
                        start=(j == 0),
                        stop=(j == j_last),
                    )

            def epilogue(qb, o_ps):
                oT = epipool.tile([DH + 1, QW], F32, tag="ot")
                nc.vector.tensor_copy(oT[:], o_ps[:])
                etp = jpool.tile([P, NQB, DH + 1], F32, tag="j", name=f"et{qb}")
                for i in range(NQB):
                    nc.tensor.transpose(
                        etp[:, i, :], oT[:, i * P:(i + 1) * P],
                        ident[:DH + 1, :DH + 1],
                    )
                recip = epipool.tile([P, NQB], F32, tag="recip")
                nc.vector.reciprocal(recip[:], etp[:, :, DH])
                o_sb = epipool.tile([P, NQB, DH], F32, tag="osb")
                for i in range(NQB):
                    nc.vector.tensor_scalar_mul(
                        o_sb[:, i, :], etp[:, i, :DH], recip[:, i:i + 1]
                    )
                nc.sync.dma_start(
                    out_d.ap()[qb * QW:(qb + 1) * QW, :].rearrange(
                        "(i p) d -> p i d", p=P
                    ),
                    o_sb[:],
                )

            # --- main emission: proj interleaved with q-block phases;
            # software-pipelined S/PV so PE work overlaps the wide exps ---
            if causal:
                proj_qk(0)
                proj_qk(1)
                for qb in range(NQB):
                    npairs = 2 * qb + 2
                    j_last = NQB * (qb + 1) - 1
                    t_projv = npairs - 1 if qb == 0 else 2 * qb
                    o_ps = opool.tile([DH + 1, QW], F32, tag="o", name=f"o{qb}")
                    p_tiles = {}
                    for t in range(npairs):
                        emit_s_pair(qb, t, p_tiles)
                        if t == t_projv:
                            proj_v(qb)
                        if t > 0:
                            emit_pv(qb, t - 1, o_ps, p_tiles, j_last)
                    emit_pv(qb, npairs - 1, o_ps, p_tiles, j_last)
                    epilogue(qb, o_ps)
                    if qb + 2 < NQB:
                        proj_qk(qb + 2)
            else:
                for s in range(NQB):
                    proj_qk(s)
                for s in range(NQB):
                    proj_v(s)
                for qb in range(NQB):
                    npairs = KC // 2
                    o_ps = opool.tile([DH + 1, QW], F32, tag="o", name=f"o{qb}")
                    p_tiles = {}
                    for t in range(npairs):
                        emit_s_pair(qb, t, p_tiles)
                        if t > 0:
                            emit_pv(qb, t - 1, o_ps, p_tiles, KC - 1)
                    emit_pv(qb, npairs - 1, o_ps, p_tiles, KC - 1)
                    epilogue(qb, o_ps)

    nc.compile()
    return nc


def _get(causal: bool, has_padding: bool):
    key = (causal, has_padding)
    if key not in _BUILD_CACHE:
        _BUILD_CACHE[key] = _build(causal, has_padding)
    return _BUILD_CACHE[key]


def _pack_x(x_t: np.ndarray, dtype) -> np.ndarray:
    """[D, N] -> SBUF tile layout [(slice p), chunk, qw]."""
    return np.ascontiguousarray(
        x_t.reshape(EC, P, NQB, QW).transpose(2, 1, 0, 3)
        .reshape(NQB * P, EC, QW).astype(dtype)
    )


def run(key_input, query_input, value_input, padding_mask, masked_attention,
        W_key, W_query=None, W_value=None, trace=False, **_ignored):
    key_input = np.asarray(key_input, dtype=np.float32)
    query_input = np.asarray(query_input, dtype=np.float32)
    value_input = np.asarray(value_input, dtype=np.float32)
    padding_mask = np.asarray(padding_mask)
    W_key = np.asarray(W_key, dtype=np.float32)

    B = key_input.shape[0]
    causal = bool(int(np.asarray(masked_attention)))
    has_padding = bool(padding_mask.any())
    nc = _get(causal, has_padding)

    bf = ml_dtypes.bfloat16
    f8 = ml_dtypes.float8_e4m3fn
    wqk = np.ascontiguousarray(
        (np.concatenate([W_key, W_key], axis=1) * WS)
        .reshape(EC, P, 2 * DH).transpose(1, 0, 2).astype(f8)
    )
    wv = np.ascontiguousarray(
        W_key.reshape(EC, P, DH).transpose(1, 0, 2).astype(bf)
    )
    in_maps = []
    for b in range(B):
        m = {
            "xq": _pack_x(query_input[b].T, f8),
            "xk": _pack_x(key_input[b].T, f8),
            "xv": _pack_x(value_input[b].T, bf),
            "wqk": wqk,
            "wv": wv,
        }
        if has_padding:
            # multiplicative key mask in [KC, P] layout: 0 where padded
            km = (~padding_mask[b].reshape(N)).astype(np.float32)
            m["kmask"] = np.ascontiguousarray(km.reshape(KC, P))
        in_maps.append(m)

    if trace:
        _ensure_ntff_hook()
    res = run_bass_kernel_spmd(nc, in_maps, core_ids=list(range(B)), trace=trace)
    out = np.stack([np.asarray(res.results[b]["out"]) for b in range(B)], axis=0)
    return out.astype(np.float32), res


def kernel(**inputs) -> np.ndarray:
    out, _ = run(**inputs)
    return out


# revision 13
# speedup vs baseline: 1.2779x; 1.1834x over previous
"""Trainium2 Bass kernel for nn_AttentionHead (B=8, N=2048, D=512, d=64).

Reference semantics (faithful to the torch original):
    K = key_input   @ W_key        # note: W_key used for Q, K AND V
    Q = query_input @ W_key
    V = value_input @ W_key
    S = Q @ K^T / sqrt(512)        # scaled by INPUT dim, not head dim
    S = mask(padding), causal-mask if masked_attention
    out = softmax(S) @ V

Sharding: pure data parallelism over batch — core b computes batch element b.
No collectives. Host-side prep is layout only (transpose + dtype cast); every
FLOP of the math runs on-device.

Device algorithm (per core), v2:
  - xq/xk stream in fp8e4 (W_qk prescaled x16 host-side, descale folded into
    the exp scale); xv stays bf16 to protect output precision
  - host packs inputs in the exact SBUF tile layout [slice*p, chunk, q] so
    every DMA line is contiguous; DMA issue is spread over 4 engine queues
    with slice-0/1 q/k prioritized
  - q-block-outer attention: per 512-wide q-block, k-chunks processed in
    row-packed pairs (two 64-deep S matmuls concurrently in disjoint PE row
    groups) writing one [128, 1024] PSUM tile; ONE wide exp per pair on ACT
    (amortizes the ~293ns ACTIVATE overhead)
  - diagonal k-chunks compute S full-width (free under row-pairing) so the
    wide exp read is contiguous; only 128-wide diagonal blocks get the
    upper-triangular mask multiply on DVE; PV uses exact causal widths
  - O.T [65, q] accumulated in PSUM over k-chunks (ones column appended to
    V-natural gives softmax denominators as row 64); per-q-block epilogue:
    PE-transpose, reciprocal, scale, batched f32 DMA out
  - PSUM: 2x [128,1024] S (4 banks) + 2x [65,512] O (2) + 2x [128,512]
    proj/transpose (2) = 8 banks
"""

import math

import numpy as np
import ml_dtypes

import concourse.bass as bass
import concourse.tile as tile
from concourse import bacc, mybir
from concourse import masks
from concourse.bass_utils import run_bass_kernel_spmd

P = 128            # partitions / k-chunk size
N = 2048           # sequence length
D = 512            # embedding dim
DH = 64            # head dim
EC = D // P        # 4 e-chunks for the projection contraction
KC = N // P        # 16 k-chunks
QW = 512           # q block width
NQB = N // QW      # 4 q blocks / n slices
WS = 16.0          # host-side W_qk prescale (fp8 range use)
SCALE = 1.0 / math.sqrt(float(D))
EXP_SCALE = SCALE / (WS * WS)

BF16 = mybir.dt.bfloat16
FP8 = mybir.dt.float8e4
F32 = mybir.dt.float32

_BUILD_CACHE = {}

OPTS = {
    "pe_warm": 12,     # dummy matmuls at t=0 to lift the HAM clock gate
    "ppool": 20,       # p_sb wide-tile buffers: one per group, so no recycle
                       # deps ever land as EVENT_SEMAPHORE waits (~222ns each)
                       # on the scalar queue between exps
}


def _ensure_ntff_hook():
    """Install the antenv.axon_hooks shim so trace=True works under axon."""
    try:
        import antenv.axon_hooks  # noqa: F401
        return
    except ImportError:
        pass
    import sys
    import types

    try:
        from trn_agent_boot.trn_boot import _ntff_profile_via_ctypes
        hook = _ntff_profile_via_ctypes("/opt/axon/libaxon_pjrt.so")
    except Exception:
        hook = None
    mod = types.ModuleType("antenv.axon_hooks")
    state = {"hook": hook}
    mod.get_axon_ntff_profile_hook = lambda: state["hook"]
    mod.set_axon_ntff_profile_hook = lambda h: state.update(hook=h)
    sys.modules["antenv.axon_hooks"] = mod
    import antenv

    antenv.axon_hooks = mod


def _build(causal: bool, has_padding: bool):
    nc = bacc.Bacc("TRN2", target_bir_lowering=False, debug=False, num_devices=8)

    # inputs prepacked host-side in SBUF tile layout [(slice p), chunk, qw]
    xq_d = nc.dram_tensor("xq", [NQB * P, EC, QW], FP8, kind="ExternalInput")
    xk_d = nc.dram_tensor("xk", [NQB * P, EC, QW], FP8, kind="ExternalInput")
    xv_d = nc.dram_tensor("xv", [NQB * P, EC, QW], BF16, kind="ExternalInput")
    # wqk is host-duplicated [W|W]*WS so Q/K projections emit [128, q] tiles
    # whose partition halves are copies — S matmul pairs then row-pack into
    # disjoint PE row groups with no cross-partition copy.
    wqk_d = nc.dram_tensor("wqk", [P, EC, 2 * DH], FP8, kind="ExternalInput")
    wv_d = nc.dram_tensor("wv", [P, EC, DH], BF16, kind="ExternalInput")
    if has_padding:
        km_d = nc.dram_tensor("kmask", [KC, P], F32, kind="ExternalInput")
    out_d = nc.dram_tensor("out", [N, DH], F32, kind="ExternalOutput")

    with tile.TileContext(nc) as tc:
        with (
            tc.tile_pool(name="const", bufs=1) as cpool,
            tc.tile_pool(name="x", bufs=8) as xpool,
            tc.tile_pool(name="big", bufs=1) as bigpool,
            tc.tile_pool(name="p", bufs=OPTS["ppool"]) as ppool,
            tc.tile_pool(name="epi", bufs=2) as epipool,
            tc.tile_pool(name="o", bufs=2, space="PSUM") as opool,
            tc.tile_pool(name="s", bufs=2, space="PSUM") as spool,
            tc.tile_pool(name="j", bufs=2, space="PSUM") as jpool,
        ):
            # --- ACT warmup: load the exp table during the DMA window ---
            warm = cpool.tile([P, 1], F32)
            nc.vector.memset(warm[:], 0.0)
            nc.scalar.activation(warm[:], warm[:], mybir.ActivationFunctionType.Exp)

            # consts emitted before any DMA issue so their engines (vector
            # memset, gpsimd affine_select) aren't stuck behind 0.7-1us
            # dma_start issue slots
            wjunk = cpool.tile([P, P], BF16)
            nc.vector.memset(wjunk[:], 0.25)
            ident = cpool.tile([P, P], F32)
            masks.make_identity(nc, ident[:])
            # upper-triangular (incl diag) 0/1 mask in [k, q] coords for the
            # causal diagonal blocks
            tri = cpool.tile([P, P], BF16)
            masks.make_upper_triangular(nc, tri[:], val=1.0, diag=True)

            # --- weights + input DMAs, spread across engine queues with
            # slice-0/1 q/k first (each dma_start costs ~0.7-1us of issue
            # time on its engine) ---
            wqk_sb = cpool.tile([P, EC, 2 * DH], FP8)
            wv_sb = cpool.tile([P, EC, DH], BF16)
            nc.scalar.dma_start(wqk_sb[:], wqk_d.ap())
            nc.gpsimd.dma_start(wv_sb[:], wv_d.ap())
            if has_padding:
                km_sb = cpool.tile([P, KC], F32)
                nc.sync.dma_start(km_sb[:], km_d.ap().transpose([1, 0]))

            # x tiles: 8 pool bufs for 16 logical tiles. Slice-2/3 tiles
            # reuse slice-0/1 buffers, which GATES their DMA start on the
            # old tile's last reader (proj matmuls) — this is the only way
            # to prioritize slice-0/1: the DMA rings round-robin across all
            # in-flight transfers, so issue order alone gives no priority.
            xq_sb, xk_sb, xv_sb = {}, {}, {}

            def dma_x(eng, t, dram, s, half=None):
                rows = dram.ap()[s * P:(s + 1) * P]
                if half is None:
                    eng.dma_start(t[:], rows)
                else:
                    eng.dma_start(
                        t[:], rows[:, half * (EC // 2):(half + 1) * (EC // 2), :]
                    )

            def alloc_dma_qk(s, eng_q, eng_k):
                xq_sb[s] = xpool.tile([P, EC, QW], FP8, tag="x", name=f"xq{s}")
                xk_sb[s] = xpool.tile([P, EC, QW], FP8, tag="x", name=f"xk{s}")
                dma_x(eng_q, xq_sb[s], xq_d, s)
                dma_x(eng_k, xk_sb[s], xk_d, s)

            def alloc_dma_v(s, eng_a, eng_b):
                xv_sb[(s, 0)] = xpool.tile([P, EC // 2, QW], BF16, tag="x",
                                           name=f"xv{s}a")
                xv_sb[(s, 1)] = xpool.tile([P, EC // 2, QW], BF16, tag="x",
                                           name=f"xv{s}b")
                dma_x(eng_a, xv_sb[(s, 0)], xv_d, s, half=0)
                dma_x(eng_b, xv_sb[(s, 1)], xv_d, s, half=1)

            # priority: q0 k0 q1 k1 | v0 v1 now; slice-2/3 tiles are
            # allocated later at reuse-gated emission points. scalar issues
            # only 3 DMAs (its exp stream must start ~10us).
            alloc_dma_qk(0, nc.scalar, nc.sync)
            alloc_dma_qk(1, nc.sync, nc.gpsimd)
            alloc_dma_v(0, nc.scalar, nc.gpsimd)
            alloc_dma_v(1, nc.sync, nc.gpsimd)

            # --- PE warmup: HAM clock-gates the PE array to 1.2 GHz until
            # ~3.4us of sustained matmul activity; spin dummy matmuls during
            # the DMA window so real work runs at 2.4 GHz ---
            if OPTS["pe_warm"]:
                wps = jpool.tile([P, QW], F32, tag="j", name="warmps")
                for _ in range(OPTS["pe_warm"]):
                    nc.tensor.matmul(
                        wps[:, :P], wjunk[:], wjunk[:],
                        start=True, stop=True, skip_group_check=True,
                    )

            qt = bigpool.tile([P, N], BF16, tag="qt")   # rows 0-63 QT, 64-127 dup
            kt = bigpool.tile([P, N], BF16, tag="kt")
            vt = bigpool.tile([DH, N], F32, tag="vt")
            v_sb = bigpool.tile([P, KC, DH + 1], BF16, tag="vn")

            def proj_qk(s):
                sl = slice(s * QW, (s + 1) * QW)
                for tname, x_t, big in (("q", xq_sb[s], qt), ("k", xk_sb[s], kt)):
                    ps = jpool.tile([P, QW], F32, tag="j", name=f"{tname}p{s}")
                    for c in range(EC):
                        nc.tensor.matmul(
                            ps[:],
                            wqk_sb[:, c, :],
                            x_t[:, c, :],
                            start=(c == 0),
                            stop=(c == EC - 1),
                        )
                    nc.vector.tensor_copy(big[:, sl], ps[:])

            def proj_v(s):
                sl = slice(s * QW, (s + 1) * QW)
                ps = jpool.tile([P, QW], F32, tag="j", name=f"vp{s}")
                for c in range(EC):
                    nc.tensor.matmul(
                        ps[:DH, :],
                        wv_sb[:, c, :],
                        xv_sb[(s, c // 2)][:, c % 2, :],
                        start=(c == 0),
                        stop=(c == EC - 1),
                    )
                nc.vector.tensor_copy(vt[:, sl], ps[:DH, :])
                # V natural tiles: PE transpose + ones column (row-sums of P
                # come free as row 64 of the PV matmul)
                vtp = jpool.tile([P, NQB, DH + 1], F32, tag="j", name=f"vt{s}")
                for i in range(NQB):
                    j = s * NQB + i
                    nc.tensor.transpose(
                        vtp[:, i, :DH], vt[:, j * P:(j + 1) * P], ident[:DH, :DH]
                    )
                nc.vector.memset(vtp[:, :, DH], 1.0)
                nc.vector.tensor_copy(v_sb[:, s * NQB:(s + 1) * NQB, :], vtp[:])

            # --- attention, q-block outer; k-chunk pairs row-packed ---
            def emit_s_pair(qb, t, p_tiles):
                """S for k-chunks (2t, 2t+1) over q-block qb: two concurrent
                row-group matmuls into one wide PSUM tile, one wide exp."""
                j0, j1 = 2 * t, 2 * t + 1
                s_ps = spool.tile([P, 2 * QW], F32, tag="s", name=f"s{qb}_{t}")
                # exp reads contiguously from q_off0; j1 computes full width
                # so no unwritten PSUM is read
                q_off0 = max(0, j0 * P - qb * QW) if causal else 0
                nc.tensor.matmul(
                    s_ps[:, q_off0:QW],
                    kt[0:DH, j0 * P:(j0 + 1) * P],
                    qt[0:DH, qb * QW + q_off0:(qb + 1) * QW],
                    start=True, stop=True,
                )
                nc.tensor.matmul(
                    s_ps[:, QW:],
                    kt[DH:P, j1 * P:(j1 + 1) * P],
                    qt[DH:P, qb * QW:(qb + 1) * QW],
                    start=True, stop=True,
                )
                p_sb = ppool.tile([P, 2 * QW], BF16, tag="p", name=f"p{qb}_{t}")
                nc.scalar.activation(
                    p_sb[:, q_off0:],
                    s_ps[:, q_off0:],
                    mybir.ActivationFunctionType.Exp,
                    scale=EXP_SCALE,
                )
                if causal:
                    # late q-blocks' diag masks go to the (by then idle)
                    # gpsimd engine to offload DVE
                    teng = nc.gpsimd if qb >= 2 else nc.vector
                    for idx, j in enumerate((j0, j1)):
                        if j // NQB == qb:
                            # diagonal 128x128 block: keep q_loc >= k_loc
                            lo = idx * QW + (j % NQB) * P
                            teng.tensor_mul(
                                p_sb[:, lo:lo + P], p_sb[:, lo:lo + P], tri[:]
                            )
                if has_padding:
                    for idx, j in enumerate((j0, j1)):
                        off = max(0, j * P - qb * QW) if causal else 0
                        nc.vector.tensor_scalar_mul(
                            p_sb[:, idx * QW + off:(idx + 1) * QW],
                            p_sb[:, idx * QW + off:(idx + 1) * QW],
                            km_sb[:, j:j + 1],
                        )
                p_tiles[t] = p_sb

            def emit_pv(qb, t, o_ps, p_tiles, j_last):
                p_sb = p_tiles.pop(t)
                for idx, j in enumerate((2 * t, 2 * t + 1)):
                    q_off = max(0, j * P - qb * QW) if causal else 0
                    nc.tensor.matmul(
                        o_ps[:, q_off:QW],
                        v_sb[:, j, :],
                        p_sb[:, idx * QW + q_off:(idx + 1) * QW],
                        start=(j == 0),
                        stop=(j == j_last),
                    )

            def epilogue(qb, o_ps):
                oT = epipool.tile([DH + 1, QW], F32, tag="ot")
                nc.vector.tensor_copy(oT[:], o_ps[:])
                etp = jpool.tile([P, NQB, DH + 1], F32, tag="j", name=f"et{qb}")
                for i in range(NQB):
                    nc.tensor.transpose(
                        etp[:, i, :], oT[:, i * P:(i + 1) * P],
                        ident[:DH + 1, :DH + 1],
                    )
                recip = epipool.tile([P, NQB], F32, tag="recip")
                nc.vector.reciprocal(recip[:], etp[:, :, DH])
                o_sb = epipool.tile([P, NQB, DH], F32, tag="osb")
                for i in range(NQB):
                    nc.vector.tensor_scalar_mul(
                        o_sb[:, i, :], etp[:, i, :DH], recip[:, i:i + 1]
                    )
                nc.sync.dma_start(
                    out_d.ap()[qb * QW:(qb + 1) * QW, :].rearrange(
                        "(i p) d -> p i d", p=P
                    ),
                    o_sb[:],
                )

            # --- main emission: proj interleaved with q-block phases;
            # software-pipelined S/PV so PE work overlaps the wide exps ---
            if causal:
                proj_qk(0)
                proj_qk(1)
                # slice-2/3 q/k tiles reuse slice-0/1 bufs: their DMA starts
                # are gated on the proj reads just emitted
                alloc_dma_qk(2, nc.sync, nc.gpsimd)
                alloc_dma_qk(3, nc.sync, nc.gpsimd)
                for qb in range(NQB):
                    npairs = 2 * qb + 2
                    j_last = NQB * (qb + 1) - 1
                    t_projv = npairs - 1 if qb == 0 else 2 * qb
                    o_ps = opool.tile([DH + 1, QW], F32, tag="o", name=f"o{qb}")
                    p_tiles = {}
                    for t in range(npairs):
                        emit_s_pair(qb, t, p_tiles)
                        if t == t_projv:
                            proj_v(qb)
                            if qb == 0:
                                alloc_dma_v(2, nc.sync, nc.gpsimd)
                            elif qb == 1:
                                alloc_dma_v(3, nc.sync, nc.gpsimd)
                        if t > 0:
                            emit_pv(qb, t - 1, o_ps, p_tiles, j_last)
                    emit_pv(qb, npairs - 1, o_ps, p_tiles, j_last)
                    epilogue(qb, o_ps)
                    if qb + 2 < NQB:
                        proj_qk(qb + 2)
            else:
                proj_qk(0)
                proj_qk(1)
                alloc_dma_qk(2, nc.sync, nc.gpsimd)
                alloc_dma_qk(3, nc.sync, nc.gpsimd)
                proj_qk(2)
                proj_qk(3)
                proj_v(0)
                proj_v(1)
                alloc_dma_v(2, nc.sync, nc.gpsimd)
                alloc_dma_v(3, nc.sync, nc.gpsimd)
                proj_v(2)
                proj_v(3)
                for qb in range(NQB):
                    npairs = KC // 2
                    o_ps = opool.tile([DH + 1, QW], F32, tag="o", name=f"o{qb}")
                    p_tiles = {}
                    for t in range(npairs):
                        emit_s_pair(qb, t, p_tiles)
                        if t > 0:
                            emit_pv(qb, t - 1, o_ps, p_tiles, KC - 1)
                    emit_pv(qb, npairs - 1, o_ps, p_tiles, KC - 1)
                    epilogue(qb, o_ps)

    nc.compile()
    return nc


def _get(causal: bool, has_padding: bool):
    key = (causal, has_padding)
    if key not in _BUILD_CACHE:
        _BUILD_CACHE[key] = _build(causal, has_padding)
    return _BUILD_CACHE[key]


def _pack_x(x_t: np.ndarray, dtype) -> np.ndarray:
    """[D, N] -> SBUF tile layout [(slice p), chunk, qw]."""
    return np.ascontiguousarray(
        x_t.reshape(EC, P, NQB, QW).transpose(2, 1, 0, 3)
        .reshape(NQB * P, EC, QW).astype(dtype)
    )


def run(key_input, query_input, value_input, padding_mask, masked_attention,
        W_key, W_query=None, W_value=None, trace=False, **_ignored):
    key_input = np.asarray(key_input, dtype=np.float32)
    query_input = np.asarray(query_input, dtype=np.float32)
    value_input = np.asarray(value_input, dtype=np.float32)
    padding_mask = np.asarray(padding_mask)
    W_key = np.asarray(W_key, dtype=np.float32)

    B = key_input.shape[0]
    causal = bool(int(np.asarray(masked_attention)))
    has_padding = bool(padding_mask.any())
    nc = _get(causal, has_padding)

    bf = ml_dtypes.bfloat16
    f8 = ml_dtypes.float8_e4m3fn
    wqk = np.ascontiguousarray(
        (np.concatenate([W_key, W_key], axis=1) * WS)
        .reshape(EC, P, 2 * DH).transpose(1, 0, 2).astype(f8)
    )
    wv = np.ascontiguousarray(
        W_key.reshape(EC, P, DH).transpose(1, 0, 2).astype(bf)
    )
    in_maps = []
    for b in range(B):
        m = {
            "xq": _pack_x(query_input[b].T, f8),
            "xk": _pack_x(key_input[b].T, f8),
            "xv": _pack_x(value_input[b].T, bf),
            "wqk": wqk,
            "wv": wv,
        }
        if has_padding:
            # multiplicative key mask in [KC, P] layout: 0 where padded
            km = (~padding_mask[b].reshape(N)).astype(np.float32)
            m["kmask"] = np.ascontiguousarray(km.reshape(KC, P))
        in_maps.append(m)

    if trace:
        _ensure_ntff_hook()
    res = run_bass_kernel_spmd(nc, in_maps, core_ids=list(range(B)), trace=trace)
    out = np.stack([np.asarray(res.results[b]["out"]) for b in range(B)], axis=0)
    return out.astype(np.float32), res


def kernel(**inputs) -> np.ndarray:
    out, _ = run(**inputs)
    return out


# revision 16
# speedup vs baseline: 1.3601x; 1.0644x over previous
"""Trainium2 Bass kernel for nn_AttentionHead (B=8, N=2048, D=512, d=64).

Reference semantics (faithful to the torch original):
    K = key_input   @ W_key        # note: W_key used for Q, K AND V
    Q = query_input @ W_key
    V = value_input @ W_key
    S = Q @ K^T / sqrt(512)        # scaled by INPUT dim, not head dim
    S = mask(padding), causal-mask if masked_attention
    out = softmax(S) @ V

Sharding: pure data parallelism over batch — core b computes batch element b.
No collectives. Host-side prep is layout only (transpose + dtype cast +
output unpermute); every FLOP of the math runs on-device.

Device algorithm (per core), v4:
  - xq/xk stream in fp8e4 (W_qk prescaled x16 host-side, descale folded into
    the exp scale); projections use DoubleRow perf mode (256-deep
    contraction, half the matmuls); xv stays bf16 to protect output precision
  - host packs inputs in the exact SBUF tile layout so every DMA line is
    contiguous; DMA transfers are sequenced into priority waves (q0k0 ->
    q1k1 -> v0 -> q2k2 -> v1 -> q3k3 -> v2 -> v3) via tiny SBUF->SBUF "gate"
    DMAs — the rings round-robin across all in-flight transfers, so issue
    order alone gives no priority
  - q-block-outer attention: per 512-wide q-block, k-chunks processed in
    row-packed pairs (two 64-deep S matmuls concurrently in disjoint PE row
    groups) writing one [128, 1024] PSUM tile; ONE wide exp per pair on ACT
    (amortizes the ~293ns ACTIVATE overhead); p_sb pool sized one-per-group
    so no recycle deps serialize the ACT queue
  - diagonal k-chunks compute S full-width (free under row-pairing) so the
    wide exp read is contiguous; only 128-wide diagonal blocks get the
    upper-triangular mask multiply (DVE early, gpsimd for late q-blocks);
    PV uses exact causal widths
  - O.T [65, q] accumulated in PSUM over k-chunks (ones column appended to
    V-natural gives softmax denominators as row 64); per-q-block epilogue:
    PE-transpose, reciprocal, then 4 independent scale->DMA chains
  - PSUM: 2x [128,1024] S (4 banks) + 2x [65,512] O (2) + 2x [128,512]
    proj/transpose (2) = 8 banks
"""

import math

import numpy as np
import ml_dtypes

import concourse.bass as bass
import concourse.tile as tile
from concourse import bacc, mybir
from concourse import masks
from concourse.bass_utils import run_bass_kernel_spmd

P = 128            # partitions / k-chunk size
N = 2048           # sequence length
D = 512            # embedding dim
DH = 64            # head dim
EC = D // P        # 4 e-chunks for the (bf16) V projection contraction
EC2 = D // (2 * P)  # 2 double-row chunks for the fp8 Q/K projections
KC = N // P        # 16 k-chunks
QW = 512           # q block width
NQB = N // QW      # 4 q blocks / n slices
WS = 16.0          # host-side W_qk prescale (fp8 range use)
SCALE = 1.0 / math.sqrt(float(D))
EXP_SCALE = SCALE / (WS * WS)

BF16 = mybir.dt.bfloat16
FP8 = mybir.dt.float8e4
F32 = mybir.dt.float32
DR = mybir.MatmulPerfMode.DoubleRow

_BUILD_CACHE = {}

OPTS = {
    "pe_warm": 12,     # dummy matmuls at t=0 to lift the HAM clock gate
    "ppool": 20,       # p_sb wide-tile buffers: one per group, so no recycle
                       # deps ever land as EVENT_SEMAPHORE waits on ACT
    "use_dr": True,    # DoubleRow fp8 projections
}


def _ensure_ntff_hook():
    """Install the antenv.axon_hooks shim so trace=True works under axon."""
    try:
        import antenv.axon_hooks  # noqa: F401
        return
    except ImportError:
        pass
    import sys
    import types

    try:
        from trn_agent_boot.trn_boot import _ntff_profile_via_ctypes
        hook = _ntff_profile_via_ctypes("/opt/axon/libaxon_pjrt.so")
    except Exception:
        hook = None
    mod = types.ModuleType("antenv.axon_hooks")
    state = {"hook": hook}
    mod.get_axon_ntff_profile_hook = lambda: state["hook"]
    mod.set_axon_ntff_profile_hook = lambda h: state.update(hook=h)
    sys.modules["antenv.axon_hooks"] = mod
    import antenv

    antenv.axon_hooks = mod


def _build(causal: bool, has_padding: bool):
    nc = bacc.Bacc("TRN2", target_bir_lowering=False, debug=False, num_devices=8)
    use_dr = OPTS["use_dr"]

    # inputs prepacked host-side in SBUF tile layout
    if use_dr:
        xq_d = nc.dram_tensor("xq", [NQB * P, EC2, 2, QW], FP8, kind="ExternalInput")
        xk_d = nc.dram_tensor("xk", [NQB * P, EC2, 2, QW], FP8, kind="ExternalInput")
        wqk_d = nc.dram_tensor("wqk", [P, EC2, 2, 2 * DH], FP8, kind="ExternalInput")
    else:
        xq_d = nc.dram_tensor("xq", [NQB * P, EC, QW], FP8, kind="ExternalInput")
        xk_d = nc.dram_tensor("xk", [NQB * P, EC, QW], FP8, kind="ExternalInput")
        wqk_d = nc.dram_tensor("wqk", [P, EC, 2 * DH], FP8, kind="ExternalInput")
    xv_d = nc.dram_tensor("xv", [NQB * P, EC, QW], BF16, kind="ExternalInput")
    wv_d = nc.dram_tensor("wv", [P, EC, DH], BF16, kind="ExternalInput")
    if has_padding:
        km_d = nc.dram_tensor("kmask", [KC, P], F32, kind="ExternalInput")
    # out rows = qb*128 + p, col block i -> full row q = qb*512 + i*128 + p
    # (host unpermutes); per-partition DMA lines are contiguous
    out_d = nc.dram_tensor("out", [NQB * P, NQB * DH], F32, kind="ExternalOutput")

    with tile.TileContext(nc) as tc:
        with (
            tc.tile_pool(name="const", bufs=1) as cpool,
            tc.tile_pool(name="x", bufs=16) as xpool,
            tc.tile_pool(name="gate", bufs=12) as gpool,
            tc.tile_pool(name="big", bufs=1) as bigpool,
            tc.tile_pool(name="p", bufs=OPTS["ppool"]) as ppool,
            tc.tile_pool(name="epi", bufs=2) as epipool,
            tc.tile_pool(name="osb", bufs=8) as opool_sb,
            tc.tile_pool(name="o", bufs=2, space="PSUM") as opool,
            tc.tile_pool(name="s", bufs=2, space="PSUM") as spool,
            tc.tile_pool(name="j", bufs=2, space="PSUM") as jpool,
        ):
            # --- ACT warmup: load the exp table during the DMA window ---
            warm = cpool.tile([P, 1], F32)
            nc.vector.memset(warm[:], 0.0)
            nc.scalar.activation(warm[:], warm[:], mybir.ActivationFunctionType.Exp)

            # consts emitted before any DMA issue so their engines (vector
            # memset, gpsimd affine_select) aren't stuck behind dma issues
            wjunk = cpool.tile([P, P], BF16)
            nc.vector.memset(wjunk[:], 0.25)
            ident = cpool.tile([P, P], F32)
            masks.make_identity(nc, ident[:])
            tri = cpool.tile([P, P], BF16)
            masks.make_upper_triangular(nc, tri[:], val=1.0, diag=True)

            # --- weights + x tiles ---
            if use_dr:
                wqk_sb = cpool.tile([P, EC2, 2, 2 * DH], FP8)
            else:
                wqk_sb = cpool.tile([P, EC, 2 * DH], FP8)
            wv_sb = cpool.tile([P, EC, DH], BF16)
            if has_padding:
                km_sb = cpool.tile([P, KC], F32)
                nc.sync.dma_start(km_sb[:], km_d.ap().transpose([1, 0]))

            xq_sb, xk_sb, xv_sb = {}, {}, {}
            for s in range(NQB):
                if use_dr:
                    xq_sb[s] = xpool.tile([P, EC2, 2, QW], FP8, tag="x",
                                          name=f"xq{s}")
                    xk_sb[s] = xpool.tile([P, EC2, 2, QW], FP8, tag="x",
                                          name=f"xk{s}")
                else:
                    xq_sb[s] = xpool.tile([P, EC, QW], FP8, tag="x", name=f"xq{s}")
                    xk_sb[s] = xpool.tile([P, EC, QW], FP8, tag="x", name=f"xk{s}")
                xv_sb[(s, 0)] = xpool.tile([P, EC // 2, QW], BF16, tag="x",
                                           name=f"xv{s}a")
                xv_sb[(s, 1)] = xpool.tile([P, EC // 2, QW], BF16, tag="x",
                                           name=f"xv{s}b")

            def dma_x(eng, t, dram, s, half=None):
                rows = dram.ap()[s * P:(s + 1) * P]
                if half is None:
                    eng.dma_start(t[:], rows)
                else:
                    eng.dma_start(
                        t[:], rows[:, half * (EC // 2):(half + 1) * (EC // 2), :]
                    )

            def gate(eng, src_tile, name):
                """Tiny SBUF->SBUF DMA that completes only after src_tile's
                DMA has fully landed — sequences the engine's next dma_start
                behind it (the rings round-robin across in-flight DMAs, so
                this is the only way to enforce transfer priority)."""
                nd = len(src_tile.shape)
                g = gpool.tile([1] * (nd - 1) + [16], src_tile.dtype,
                               tag="g", name=name)
                sl = tuple([slice(0, 1)] * (nd - 1) + [slice(0, 16)])
                eng.dma_start(g[:], src_tile[sl])

            # wave 0: weights + q0/k0 (+ q1 on the third queue)
            nc.scalar.dma_start(wqk_sb[:], wqk_d.ap())
            dma_x(nc.scalar, xq_sb[0], xq_d, 0)
            dma_x(nc.sync, xk_sb[0], xk_d, 0)
            nc.gpsimd.dma_start(wv_sb[:], wv_d.ap())
            # sync ladder: q1 -> v0a -> q2 -> v1a -> q3 -> v2a -> v3a
            gate(nc.sync, xk_sb[0], "gs0")
            dma_x(nc.sync, xq_sb[1], xq_d, 1)
            gate(nc.sync, xq_sb[1], "gs1")
            dma_x(nc.sync, xv_sb[(0, 0)], xv_d, 0, half=0)
            gate(nc.sync, xv_sb[(0, 0)], "gs2")
            dma_x(nc.sync, xq_sb[2], xq_d, 2)
            gate(nc.sync, xq_sb[2], "gs3")
            dma_x(nc.sync, xv_sb[(1, 0)], xv_d, 1, half=0)
            gate(nc.sync, xv_sb[(1, 0)], "gs4")
            dma_x(nc.sync, xq_sb[3], xq_d, 3)
            gate(nc.sync, xq_sb[3], "gs5")
            dma_x(nc.sync, xv_sb[(2, 0)], xv_d, 2, half=0)
            gate(nc.sync, xv_sb[(2, 0)], "gs6")
            dma_x(nc.sync, xv_sb[(3, 0)], xv_d, 3, half=0)
            # gpsimd ladder: k1 -> v0b -> k2 -> v1b -> k3 -> v2b -> v3b
            gate(nc.gpsimd, xk_sb[0], "gg0")
            dma_x(nc.gpsimd, xk_sb[1], xk_d, 1)
            gate(nc.gpsimd, xk_sb[1], "gg1")
            dma_x(nc.gpsimd, xv_sb[(0, 1)], xv_d, 0, half=1)
            gate(nc.gpsimd, xv_sb[(0, 1)], "gg2")
            dma_x(nc.gpsimd, xk_sb[2], xk_d, 2)
            gate(nc.gpsimd, xk_sb[2], "gg3")
            dma_x(nc.gpsimd, xv_sb[(1, 1)], xv_d, 1, half=1)
            gate(nc.gpsimd, xv_sb[(1, 1)], "gg4")
            dma_x(nc.gpsimd, xk_sb[3], xk_d, 3)
            gate(nc.gpsimd, xk_sb[3], "gg5")
            dma_x(nc.gpsimd, xv_sb[(2, 1)], xv_d, 2, half=1)
            gate(nc.gpsimd, xv_sb[(2, 1)], "gg6")
            dma_x(nc.gpsimd, xv_sb[(3, 1)], xv_d, 3, half=1)

            # --- PE warmup: HAM clock-gates the PE array to 1.2 GHz until
            # ~3.4us of sustained matmul activity ---
            if OPTS["pe_warm"]:
                wps = jpool.tile([P, QW], F32, tag="j", name="warmps")
                for _ in range(OPTS["pe_warm"]):
                    nc.tensor.matmul(
                        wps[:, :P], wjunk[:], wjunk[:],
                        start=True, stop=True, skip_group_check=True,
                    )

            qt = bigpool.tile([P, N], BF16, tag="qt")   # rows 0-63 QT, 64-127 dup
            kt = bigpool.tile([P, N], BF16, tag="kt")
            vt = bigpool.tile([DH, N], F32, tag="vt")
            v_sb = bigpool.tile([P, KC, DH + 1], BF16, tag="vn")

            def proj_qk(s):
                sl = slice(s * QW, (s + 1) * QW)
                for tname, x_t, big in (("q", xq_sb[s], qt), ("k", xk_sb[s], kt)):
                    ps = jpool.tile([P, QW], F32, tag="j", name=f"{tname}p{s}")
                    if use_dr:
                        for c in range(EC2):
                            nc.tensor.matmul(
                                ps[:],
                                wqk_sb[:, c],
                                x_t[:, c],
                                start=(c == 0),
                                stop=(c == EC2 - 1),
                                perf_mode=DR,
                            )
                    else:
                        for c in range(EC):
                            nc.tensor.matmul(
                                ps[:],
                                wqk_sb[:, c, :],
                                x_t[:, c, :],
                                start=(c == 0),
                                stop=(c == EC - 1),
                            )
                    nc.vector.tensor_copy(big[:, sl], ps[:])

            def proj_v(s):
                sl = slice(s * QW, (s + 1) * QW)
                ps = jpool.tile([P, QW], F32, tag="j", name=f"vp{s}")
                for c in range(EC):
                    nc.tensor.matmul(
                        ps[:DH, :],
                        wv_sb[:, c, :],
                        xv_sb[(s, c // 2)][:, c % 2, :],
                        start=(c == 0),
                        stop=(c == EC - 1),
                    )
                nc.vector.tensor_copy(vt[:, sl], ps[:DH, :])
                # V natural tiles: PE transpose + ones column (row-sums of P
                # come free as row 64 of the PV matmul)
                vtp = jpool.tile([P, NQB, DH + 1], F32, tag="j", name=f"vt{s}")
                for i in range(NQB):
                    j = s * NQB + i
                    nc.tensor.transpose(
                        vtp[:, i, :DH], vt[:, j * P:(j + 1) * P], ident[:DH, :DH]
                    )
                nc.vector.memset(vtp[:, :, DH], 1.0)
                nc.vector.tensor_copy(v_sb[:, s * NQB:(s + 1) * NQB, :], vtp[:])

            # --- attention, q-block outer; k-chunk pairs row-packed ---
            def emit_s_pair(qb, t, p_tiles):
                j0, j1 = 2 * t, 2 * t + 1
                s_ps = spool.tile([P, 2 * QW], F32, tag="s", name=f"s{qb}_{t}")
                # exp reads contiguously from q_off0; j1 computes full width
                # so no unwritten PSUM is read
                q_off0 = max(0, j0 * P - qb * QW) if causal else 0
                nc.tensor.matmul(
                    s_ps[:, q_off0:QW],
                    kt[0:DH, j0 * P:(j0 + 1) * P],
                    qt[0:DH, qb * QW + q_off0:(qb + 1) * QW],
                    start=True, stop=True,
                )
                nc.tensor.matmul(
                    s_ps[:, QW:],
                    kt[DH:P, j1 * P:(j1 + 1) * P],
                    qt[DH:P, qb * QW:(qb + 1) * QW],
                    start=True, stop=True,
                )
                p_sb = ppool.tile([P, 2 * QW], BF16, tag="p", name=f"p{qb}_{t}")
                nc.scalar.activation(
                    p_sb[:, q_off0:],
                    s_ps[:, q_off0:],
                    mybir.ActivationFunctionType.Exp,
                    scale=EXP_SCALE,
                )
                if causal:
                    # late q-blocks' diag masks go to the (by then idle)
                    # gpsimd engine to offload DVE
                    teng = nc.gpsimd if qb >= 2 else nc.vector
                    for idx, j in enumerate((j0, j1)):
                        if j // NQB == qb:
                            # diagonal 128x128 block: keep q_loc >= k_loc
                            lo = idx * QW + (j % NQB) * P
                            teng.tensor_mul(
                                p_sb[:, lo:lo + P], p_sb[:, lo:lo + P], tri[:]
                            )
                if has_padding:
                    for idx, j in enumerate((j0, j1)):
                        off = max(0, j * P - qb * QW) if causal else 0
                        nc.vector.tensor_scalar_mul(
                            p_sb[:, idx * QW + off:(idx + 1) * QW],
                            p_sb[:, idx * QW + off:(idx + 1) * QW],
                            km_sb[:, j:j + 1],
                        )
                p_tiles[t] = p_sb

            def emit_pv(qb, t, o_ps, p_tiles, j_last):
                p_sb = p_tiles.pop(t)
                for idx, j in enumerate((2 * t, 2 * t + 1)):
                    q_off = max(0, j * P - qb * QW) if causal else 0
                    nc.tensor.matmul(
                        o_ps[:, q_off:QW],
                        v_sb[:, j, :],
                        p_sb[:, idx * QW + q_off:(idx + 1) * QW],
                        start=(j == 0),
                        stop=(j == j_last),
                    )

            def epilogue(qb, o_ps):
                oT = epipool.tile([DH + 1, QW], F32, tag="ot")
                nc.vector.tensor_copy(oT[:], o_ps[:])
                etp = jpool.tile([P, NQB, DH + 1], F32, tag="j", name=f"et{qb}")
                for i in range(NQB):
                    nc.tensor.transpose(
                        etp[:, i, :], oT[:, i * P:(i + 1) * P],
                        ident[:DH + 1, :DH + 1],
                    )
                recip = epipool.tile([P, NQB], F32, tag="recip")
                nc.vector.reciprocal(recip[:], etp[:, :, DH])
                # 4 independent scale->DMA chains (separate tiles so no
                # false WAW serialization; contiguous 256B lines per i)
                for i in range(NQB):
                    o_sb = opool_sb.tile([P, DH], F32, tag="osb",
                                         name=f"osb{qb}_{i}")
                    nc.vector.tensor_scalar_mul(
                        o_sb[:], etp[:, i, :DH], recip[:, i:i + 1]
                    )
                    nc.sync.dma_start(
                        out_d.ap()[qb * P:(qb + 1) * P, i * DH:(i + 1) * DH],
                        o_sb[:],
                    )

            # --- main emission: proj interleaved with q-block phases;
            # software-pipelined S/PV so PE work overlaps the wide exps ---
            if causal:
                proj_qk(0)
                proj_qk(1)
                for qb in range(NQB):
                    npairs = 2 * qb + 2
                    j_last = NQB * (qb + 1) - 1
                    t_projv = npairs - 1 if qb == 0 else 2 * qb
                    o_ps = opool.tile([DH + 1, QW], F32, tag="o", name=f"o{qb}")
                    p_tiles = {}
                    for t in range(npairs):
                        emit_s_pair(qb, t, p_tiles)
                        if t == t_projv:
                            proj_v(qb)
                        if t > 0:
                            emit_pv(qb, t - 1, o_ps, p_tiles, j_last)
                    emit_pv(qb, npairs - 1, o_ps, p_tiles, j_last)
                    epilogue(qb, o_ps)
                    if qb + 2 < NQB:
                        proj_qk(qb + 2)
            else:
                for s in range(NQB):
                    proj_qk(s)
                for s in range(NQB):
                    proj_v(s)
                for qb in range(NQB):
                    npairs = KC // 2
                    o_ps = opool.tile([DH + 1, QW], F32, tag="o", name=f"o{qb}")
                    p_tiles = {}
                    for t in range(npairs):
                        emit_s_pair(qb, t, p_tiles)
                        if t > 0:
                            emit_pv(qb, t - 1, o_ps, p_tiles, KC - 1)
                    emit_pv(qb, npairs - 1, o_ps, p_tiles, KC - 1)
                    epilogue(qb, o_ps)

    nc.compile()
    return nc


def _get(causal: bool, has_padding: bool):
    key = (causal, has_padding)
    if key not in _BUILD_CACHE:
        _BUILD_CACHE[key] = _build(causal, has_padding)
    return _BUILD_CACHE[key]


def _pack_x(x_t: np.ndarray, dtype) -> np.ndarray:
    """[D, N] -> SBUF tile layout [(slice p), chunk, qw]."""
    return np.ascontiguousarray(
        x_t.reshape(EC, P, NQB, QW).transpose(2, 1, 0, 3)
        .reshape(NQB * P, EC, QW).astype(dtype)
    )


def _pack_x_dr(x_t: np.ndarray, dtype) -> np.ndarray:
    """[D, N] -> DoubleRow tile layout [(slice p), c, ko, qw],
    d = c*256 + ko*128 + ki."""
    return np.ascontiguousarray(
        x_t.reshape(EC2, 2, P, NQB, QW).transpose(3, 2, 0, 1, 4)
        .reshape(NQB * P, EC2, 2, QW).astype(dtype)
    )


def run(key_input, query_input, value_input, padding_mask, masked_attention,
        W_key, W_query=None, W_value=None, trace=False, **_ignored):
    key_input = np.asarray(key_input, dtype=np.float32)
    query_input = np.asarray(query_input, dtype=np.float32)
    value_input = np.asarray(value_input, dtype=np.float32)
    padding_mask = np.asarray(padding_mask)
    W_key = np.asarray(W_key, dtype=np.float32)

    B = key_input.shape[0]
    causal = bool(int(np.asarray(masked_attention)))
    has_padding = bool(padding_mask.any())
    nc = _get(causal, has_padding)

    bf = ml_dtypes.bfloat16
    f8 = ml_dtypes.float8_e4m3fn
    wcat = np.concatenate([W_key, W_key], axis=1) * WS
    if OPTS["use_dr"]:
        wqk = np.ascontiguousarray(
            wcat.reshape(EC2, 2, P, 2 * DH).transpose(2, 0, 1, 3).astype(f8)
        )
    else:
        wqk = np.ascontiguousarray(
            wcat.reshape(EC, P, 2 * DH).transpose(1, 0, 2).astype(f8)
        )
    wv = np.ascontiguousarray(
        W_key.reshape(EC, P, DH).transpose(1, 0, 2).astype(bf)
    )
    pack_qk = _pack_x_dr if OPTS["use_dr"] else _pack_x
    in_maps = []
    for b in range(B):
        m = {
            "xq": pack_qk(query_input[b].T, f8),
            "xk": pack_qk(key_input[b].T, f8),
            "xv": _pack_x(value_input[b].T, bf),
            "wqk": wqk,
            "wv": wv,
        }
        if has_padding:
            km = (~padding_mask[b].reshape(N)).astype(np.float32)
            m["kmask"] = np.ascontiguousarray(km.reshape(KC, P))
        in_maps.append(m)

    if trace:
        _ensure_ntff_hook()
    res = run_bass_kernel_spmd(nc, in_maps, core_ids=list(range(B)), trace=trace)
    outs = []
    for b in range(B):
        o = np.asarray(res.results[b]["out"])  # [(qb p), (i d)]
        o = o.reshape(NQB, P, NQB, DH).transpose(0, 2, 1, 3).reshape(N, DH)
        outs.append(o)
    out = np.stack(outs, axis=0)
    return out.astype(np.float32), res


def kernel(**inputs) -> np.ndarray:
    out, _ = run(**inputs)
    return out
